# revision 18
# baseline (speedup 1.0000x reference)
"""Causal self-attention (b=2, s=2048, d=1024, h=16) on 8 TRN2 NeuronCores.

Sharding: batch x head-group (2 x 4). Core c handles batch c//4 and heads
[4*(c%4), 4*(c%4)+4). W_qkv is column-sharded / W_proj row-sharded over
heads; each core produces a partial [2048, 1024] projection output and the
host sums the 4 partials per batch (the all-reduce of the hinted TP layout,
done during unshard).

Per-core kernel (all matmuls fp16 with fp32 PSUM accumulation):
  qkvT = Wg.T @ x.T       -> qT,kT [dh-on-partition] and v [s, dh] (+ones col)
  S.T[j,i] = k.T q        -> PT = exp(S.T/8) via ACT, fp16; causal handled by
                             computing only j<=i tiles + masking diag blocks
  yT[dh,i] = (v|1).T @ PT -> row 64 is the softmax denominator
  yT *= bcast(1/den)      -> DMA partition-broadcast + DVE multiply
  out[i,:] += yT.T W_proj -> fp16 matmul, fp32 out
"""
import sys
import types
from contextlib import ExitStack

import numpy as np

# ── environment shims ──────────────────────────────────────────────────
# 1) antenv.axon_hooks is missing from this image; bass_utils imports it
#    when tracing is requested (including via the BASS_TRACE env var).
# 2) upload_artifacts needs an S3 bucket this container doesn't have.


def _install_shims():
    if "antenv.axon_hooks" not in sys.modules:
        try:
            import antenv  # noqa: F401

            _m = types.ModuleType("antenv.axon_hooks")
            _m._hook = None
            _m.set_axon_ntff_profile_hook = lambda h: setattr(_m, "_hook", h)
            _m.get_axon_ntff_profile_hook = lambda: _m._hook
            sys.modules["antenv.axon_hooks"] = _m
            from trn_agent_boot.trn_boot import _ntff_profile_via_ctypes

            _m.set_axon_ntff_profile_hook(
                _ntff_profile_via_ctypes("/opt/axon/libaxon_pjrt.so")
            )
        except Exception:
            pass
    try:
        import concourse.bass_utils as _bu

        _orig = _bu.upload_artifacts

        def _safe_upload(tmpdir):
            try:
                return _orig(tmpdir)
            except Exception:
                return tmpdir

        _bu.upload_artifacts = _safe_upload
    except Exception:
        pass


_install_shims()

import concourse.bass as bass  # noqa: E402
import concourse.tile as tile  # noqa: E402
from concourse import mybir  # noqa: E402
from concourse.bass_utils import run_bass_kernel_spmd  # noqa: E402
from concourse.vector_clock import ScopedClock  # noqa: E402

FP16 = mybir.dt.float16
FP32 = mybir.dt.float32
AF = mybir.ActivationFunctionType

B, S, D = 2, 2048, 1024
NH, DH = 16, 64
HPC = 4          # heads per core
N_CORES = 8
NT = S // 128    # 16 i/j tiles
NCH = S // 512   # 4 s-chunks


class FixedTileContext(tile.TileContext):
    """Workaround: this walrus build rejects >1 sync-wait command on any
    instruction ("Too many sync wait commands"). After Tile assigns waits,
    rewrite every instruction carrying N>1 waits into N-1 single-wait
    same-engine nops placed immediately before it (engine queues are
    in-order, so the semantics are identical)."""

    def _split_multiwaits(self):
        nc = self.nc
        blocks = nc.m.functions[0].blocks
        work = []
        for block in blocks:
            for inst in block.instructions:
                si = inst.sync_info
                if si is not None and si.on_wait and len(si.on_wait) > 1:
                    work.append(inst)
        if not work:
            return
        # Create the helper nops (they append to cur_bb; we pull them out).
        nop_map = {}
        created = []
        for inst in work:
            waits = list(inst.sync_info.on_wait)
            nops = []
            for w in waits[:-1]:
                bi = nc.engines[inst.engine].nop(nofuse=True)
                nsi = bi.ins.sync_info
                if nsi is None:
                    bi.ins.sync_info = type(inst.sync_info)(
                        on_wait=[w], on_update=[]
                    )
                else:
                    nsi.on_wait = [w]
                nops.append(bi.ins)
                created.append(bi.ins)
            inst.sync_info.on_wait = waits[-1:]
            nop_map[inst.name] = nops
        created_names = {i.name for i in created}
        for block in blocks:
            insts = block.instructions
            if not any(i.name in nop_map or i.name in created_names
                       for i in insts):
                continue
            new = []
            for inst in insts:
                if inst.name in created_names:
                    continue  # strip from wherever nop() appended it
                new.extend(nop_map.get(inst.name, ()))
                new.append(inst)
            block.instructions[:] = new

    def _drain_and_barrier(self, tick_clock, wait_clock):
        nc = self.nc
        drain_inst = nc.sync.drain()
        wait_clock.add_sem_waits(
            drain_inst.ins, ScopedClock({None: tick_clock.global_clock})
        )
        nc.all_engine_barrier()
        assert self.sems is not None
        popped = nc._tile_sem_poison_stack.pop()
        assert popped is self._sem_poison
        nc.clear_and_free_semaphores(list(self.sems.allocated().values()))
        nc.all_engine_barrier()
        self._split_multiwaits()


def build_core_kernel():
    nc = bass.Bass(
        trn_type="TRN2", target_bir_lowering=False, debug=False,
        num_devices=N_CORES,
    )
    xT = nc.dram_tensor("xT", [D, S], FP16, kind="ExternalInput").ap()
    wqkv = nc.dram_tensor("wqkv", [D, 3 * HPC * DH], FP16, kind="ExternalInput").ap()
    wproj = nc.dram_tensor("wproj", [HPC * DH, D], FP16, kind="ExternalInput").ap()
    tri = nc.dram_tensor("tri", [128, 128], FP16, kind="ExternalInput").ap()
    out = nc.dram_tensor("out", [S, D], FP32, kind="ExternalOutput").ap()

    with FixedTileContext(nc) as tc, ExitStack() as ctx:
        p_w = ctx.enter_context(tc.tile_pool(name="w", bufs=1))
        p_xt = ctx.enter_context(tc.tile_pool(name="xt", bufs=1))
        p_qk = ctx.enter_context(tc.tile_pool(name="qk", bufs=1))
        p_v = ctx.enter_context(tc.tile_pool(name="v", bufs=1))
        p_pt = ctx.enter_context(tc.tile_pool(name="pt", bufs=1))
        p_yt = ctx.enter_context(tc.tile_pool(name="yt", bufs=1))
        p_den = ctx.enter_context(tc.tile_pool(name="den", bufs=2))
        p_ob = ctx.enter_context(tc.tile_pool(name="ob", bufs=3))
        p_ps_mm = ctx.enter_context(tc.tile_pool(name="ps_mm", bufs=2, space="PSUM"))
        p_ps_s = ctx.enter_context(tc.tile_pool(name="ps_s", bufs=2, space="PSUM"))
        p_ps_pv = ctx.enter_context(tc.tile_pool(name="ps_pv", bufs=2, space="PSUM"))

        # ── weights / constants ────────────────────────────────────────
        w_tiles = []
        for k in range(8):
            t = p_w.tile([128, 3 * HPC * DH], FP16, tag=f"wqkv{k}", name=f"wqkv{k}")
            nc.sync.dma_start(t[:], wqkv[k * 128:(k + 1) * 128, :])
            w_tiles.append(t)
        wp_tiles = []
        for k in range(2):
            t = p_w.tile([128, D], FP16, tag=f"wp{k}", name=f"wp{k}")
            nc.sync.dma_start(t[:], wproj[k * 128:(k + 1) * 128, :])
            wp_tiles.append(t)
        tri_t = p_w.tile([128, 128], FP16, tag="tri")
        nc.sync.dma_start(tri_t[:], tri[:, :])
        ones_row = p_w.tile([1, 64], FP16, tag="ones", name="ones_row")
        nc.vector.memset(ones_row[:], 1.0)

        # ── persistent activations ─────────────────────────────────────
        # qkT[m][p, s]: m=0,1 -> qT head pairs (2h, 2h+1), m=2,3 -> kT
        qk_tiles = [p_qk.tile([128, S], FP16, tag=f"qk{m}", name=f"qk{m}")
                    for m in range(4)]
        # v_aug[J][p=j, 4*65]: per head 64 v cols + ones col (denominator)
        v_tiles = [p_v.tile([128, HPC * 65], FP16, tag=f"v{j}", name=f"v{j}")
                   for j in range(NT)]
        # yT head pairs [128 = 2 heads x 64, S]
        yt_tiles = [p_yt.tile([128, S], FP16, tag=f"yt{m}", name=f"yt{m}")
                    for m in range(2)]

        # ── phase A: qkv projection (contraction over d on partitions) ──
        # xT resident as 8 [128, 2048] fp16 tiles; emission interleaves the
        # qkv sub-phases with the first heads' score computation so ACT
        # starts early.
        xt_tiles = []
        for k in range(8):
            t = p_xt.tile([128, S], FP16, tag=f"xt{k}", name=f"xt{k}")
            nc.sync.dma_start(t[:], xT[k * 128:(k + 1) * 128, :])
            xt_tiles.append(t)

        def qk_phase(ms):
            """qT/kT m-tiles: out[c-tile 128, s 512] = W[:, cslice].T @ xT."""
            for sc in range(NCH):
                for m in ms:
                    ps = p_ps_mm.tile([128, 512], FP32, tag="ps_mm",
                                      name="ps_mm")
                    for k in range(8):
                        nc.tensor.matmul(
                            ps[:],
                            lhsT=w_tiles[k][:, m * 128:(m + 1) * 128],
                            rhs=xt_tiles[k][:, sc * 512:(sc + 1) * 512],
                            start=(k == 0), stop=(k == 7),
                        )
                    nc.vector.tensor_copy(
                        qk_tiles[m][:, sc * 512:(sc + 1) * 512], ps[:]
                    )

        def v_phase():
            """v: out[s-tile 128, 4*64] = xT[:, stile].T @ Wv, + ones col."""
            for j in range(NT):
                ps = p_ps_mm.tile([128, 256], FP32, tag="ps_mm", name="ps_mm")
                for k in range(8):
                    nc.tensor.matmul(
                        ps[:],
                        lhsT=xt_tiles[k][:, j * 128:(j + 1) * 128],
                        rhs=w_tiles[k][:, 2 * HPC * DH:3 * HPC * DH],
                        start=(k == 0), stop=(k == 7),
                    )
                va3 = v_tiles[j][:].rearrange("p (h c) -> p h c", c=65)
                nc.vector.memset(va3[:, :, 64:65], 1.0)
                nc.vector.tensor_copy(
                    va3[:, :, 0:64],
                    ps[:].rearrange("p (h c) -> p h c", c=64),
                )

        # ── phases B/C: attention ──────────────────────────────────────
        # PT tiles: [128=j, width] fp16, width = S - 128*J
        pt_tiles = {}

        def st_exp(h):
            """S.T + exp for head h: PT[J][j, i-i0] = exp(qk/8), causal.
            exp ops are 1024 wide (2 psum banks) to amortize ACT bubbles;
            diagonal-block causal masking runs on the idle Pool engine."""
            par = h % 2
            qt = qk_tiles[h // 2]
            kt = qk_tiles[2 + h // 2]
            for J in range(NT):
                i0 = J * 128
                width = S - i0
                pt = p_pt.tile([128, width], FP16, tag=f"pt{J}_{par}",
                               name=f"pt{h}_{J}")
                pt_tiles[(h, J)] = pt
                for t in range((width + 1023) // 1024):
                    n = min(1024, width - t * 1024)
                    ps = p_ps_s.tile([128, 1024], FP32, tag="ps_s",
                                     name="ps_s")
                    for u in range((n + 511) // 512):
                        nu = min(512, n - u * 512)
                        nc.tensor.matmul(
                            ps[:, u * 512:u * 512 + nu],
                            lhsT=kt[par * 64:(par + 1) * 64, i0:i0 + 128],
                            rhs=qt[par * 64:(par + 1) * 64,
                                   i0 + t * 1024 + u * 512:
                                   i0 + t * 1024 + u * 512 + nu],
                            start=True, stop=True,
                        )
                    nc.scalar.activation(
                        pt[:, t * 1024:t * 1024 + n], ps[:, 0:n], AF.Exp,
                        scale=0.125,
                    )
                    if t == 0:
                        # causal mask on the diagonal 128x128 block
                        nc.gpsimd.tensor_mul(pt[:, 0:128], pt[:, 0:128],
                                             tri_t[:])

        def pv_norm(h, tail_cb=None):
            """PV + normalization for head h, 512-wide chunks.

            The denominator broadcast (ones[64].T @ recip row, a K=1 matmul)
            is emitted one chunk late so PE never waits on the DVE recip."""
            par = h % 2
            yt = yt_tiles[h // 2]
            denr = p_den.tile([1, S], FP16, tag="denr", name=f"denr{h}")
            denb = p_den.tile([64, S], FP32, tag="denb", name=f"denb{h}")

            def finish(q):
                i0 = 512 * q
                ps = pss[q]
                psb = p_ps_s.tile([64, 512], FP32, tag="ps_s", name="ps_bc")
                nc.tensor.matmul(
                    psb[:], lhsT=ones_row[0:1, :], rhs=denr[0:1, i0:i0 + 512],
                    start=True, stop=True,
                )
                nc.vector.tensor_copy(denb[:, i0:i0 + 512], psb[:])
                nc.vector.tensor_mul(
                    yt[par * 64:(par + 1) * 64, i0:i0 + 512],
                    ps[0:64, :],
                    denb[:, i0:i0 + 512],
                )
                if tail_cb is not None:
                    tail_cb(q)

            pss = {}
            for q in range(NT // 4):
                i0 = 512 * q
                last_J = 4 * q + 3
                ps = p_ps_pv.tile([65, 512], FP32, tag="ps_pv", name="ps_pv")
                pss[q] = ps
                for J in range(last_J + 1):
                    off = max(0, 128 * J - i0)
                    src = i0 + off - 128 * J
                    nc.tensor.matmul(
                        ps[:, off:512],
                        lhsT=v_tiles[J][:, h * 65:(h + 1) * 65],
                        rhs=pt_tiles[(h, J)][:, src:src + 512 - off],
                        start=(J == 0), stop=(J == last_J),
                    )
                with nc.allow_low_precision("softmax denom recip in fp16"):
                    nc.vector.reciprocal(denr[0:1, i0:i0 + 512], ps[64:65, :])
                if q > 0:
                    finish(q - 1)
            finish(NT // 4 - 1)
            for J in range(NT):
                del pt_tiles[(h, J)]

        # ── phase D: output projection (partial over this core's heads) ─
        def proj(q):
            for it in range(4 * q, 4 * q + 4):
                for n2 in range(2):
                    ps = p_ps_mm.tile([128, 512], FP32, tag="ps_mm",
                                      name="ps_mm")
                    for kc in range(2):
                        nc.tensor.matmul(
                            ps[:],
                            lhsT=yt_tiles[kc][:, it * 128:(it + 1) * 128],
                            rhs=wp_tiles[kc][:, n2 * 512:(n2 + 1) * 512],
                            start=(kc == 0), stop=(kc == 1),
                        )
                    ob = p_ob.tile([128, 512], FP32, tag="ob", name="ob")
                    nc.scalar.copy(ob[:], ps[:])
                    nc.sync.dma_start(
                        out[it * 128:(it + 1) * 128,
                            n2 * 512:(n2 + 1) * 512], ob[:]
                    )

        # ── emission schedule (PE program order == emission order) ─────
        qk_phase([0, 2])     # q/k for heads 0,1
        st_exp(0)
        v_phase()
        st_exp(1)
        qk_phase([1, 3])     # q/k for heads 2,3
        pv_norm(0)
        st_exp(2)
        pv_norm(1)
        st_exp(3)
        pv_norm(2)
        pv_norm(3, tail_cb=proj)

    return nc


def make_in_maps(x, W_qkv, W_proj):
    tri = np.triu(np.ones((128, 128), dtype=np.float16))
    in_maps = []
    for c in range(N_CORES):
        b, g = c // 4, c % 4
        h0 = g * HPC
        cols = slice(h0 * DH, (h0 + HPC) * DH)
        wg = np.concatenate(
            [W_qkv[:, 0 * D:][:, cols], W_qkv[:, 1 * D:][:, cols],
             W_qkv[:, 2 * D:][:, cols]], axis=1,
        ).astype(np.float16)
        in_maps.append({
            "xT": np.ascontiguousarray(x[b].T).astype(np.float16),
            "wqkv": np.ascontiguousarray(wg),
            "wproj": np.ascontiguousarray(W_proj[cols, :]).astype(np.float16),
            "tri": tri,
        })
    return in_maps


_NC_CACHE = None


def run(x, W_qkv, W_proj, trace=False):
    global _NC_CACHE
    if _NC_CACHE is None:
        _NC_CACHE = build_core_kernel()
    nc = _NC_CACHE
    in_maps = make_in_maps(x, W_qkv, W_proj)
    res = run_bass_kernel_spmd(nc, in_maps, list(range(N_CORES)), trace=trace)
    outs = [res.results[c]["out"] for c in range(N_CORES)]
    full = np.stack(
        [outs[4 * b] + outs[4 * b + 1] + outs[4 * b + 2] + outs[4 * b + 3]
         for b in range(B)]
    ).astype(np.float32)
    return full, res


def kernel(x, W_qkv, W_proj):
    full, _ = run(np.asarray(x), np.asarray(W_qkv), np.asarray(W_proj))
    return full


# revision 25
# speedup vs baseline: 1.3233x; 1.3233x over previous
"""Causal self-attention (b=2, s=2048, d=1024, h=16) on 8 TRN2 NeuronCores.

Sharding: batch x head-group (2 x 4). Core c handles batch c//4 and heads
[4*(c%4), 4*(c%4)+4). W_qkv is column-sharded / W_proj row-sharded over
heads; each core produces a partial [2048, 1024] projection output and the
host sums the 4 partials per batch (the all-reduce of the hinted TP layout,
done during unshard).

Per-core kernel (all matmuls fp16 with fp32 PSUM accumulation):
  qkvT = Wg.T @ x.T       -> qT,kT [dh-on-partition] and v [s, dh] (+ones col)
  S.T[j,i] = k.T q        -> PT = exp(S.T/8) via ACT, fp16; causal handled by
                             computing only j<=i tiles + masking diag blocks
  yT[dh,i] = (v|1).T @ PT -> row 64 is the softmax denominator
  yT *= bcast(1/den)      -> DMA partition-broadcast + DVE multiply
  out[i,:] += yT.T W_proj -> fp16 matmul, fp32 out
"""
import sys
import types
from contextlib import ExitStack

import numpy as np

# ── environment shims ──────────────────────────────────────────────────
# 1) antenv.axon_hooks is missing from this image; bass_utils imports it
#    when tracing is requested (including via the BASS_TRACE env var).
# 2) upload_artifacts needs an S3 bucket this container doesn't have.


def _install_shims():
    if "antenv.axon_hooks" not in sys.modules:
        try:
            import antenv  # noqa: F401

            _m = types.ModuleType("antenv.axon_hooks")
            _m._hook = None
            _m.set_axon_ntff_profile_hook = lambda h: setattr(_m, "_hook", h)
            _m.get_axon_ntff_profile_hook = lambda: _m._hook
            sys.modules["antenv.axon_hooks"] = _m
            from trn_agent_boot.trn_boot import _ntff_profile_via_ctypes

            _m.set_axon_ntff_profile_hook(
                _ntff_profile_via_ctypes("/opt/axon/libaxon_pjrt.so")
            )
        except Exception:
            pass
    try:
        import concourse.bass_utils as _bu

        _orig = _bu.upload_artifacts

        def _safe_upload(tmpdir):
            try:
                return _orig(tmpdir)
            except Exception:
                return tmpdir

        _bu.upload_artifacts = _safe_upload
    except Exception:
        pass


_install_shims()

import concourse.bass as bass  # noqa: E402
import concourse.tile as tile  # noqa: E402
from concourse import mybir  # noqa: E402
from concourse.bass_utils import run_bass_kernel_spmd  # noqa: E402
from concourse.vector_clock import ScopedClock  # noqa: E402

FP16 = mybir.dt.float16
FP32 = mybir.dt.float32
AF = mybir.ActivationFunctionType

B, S, D = 2, 2048, 1024
NH, DH = 16, 64
HPC = 4          # heads per core
N_CORES = 8
NT = S // 128    # 16 i/j tiles
NCH = S // 512   # 4 s-chunks


class FixedTileContext(tile.TileContext):
    """Workaround: this walrus build rejects >1 sync-wait command on any
    instruction ("Too many sync wait commands"). After Tile assigns waits,
    rewrite every instruction carrying N>1 waits into N-1 single-wait
    same-engine nops placed immediately before it (engine queues are
    in-order, so the semantics are identical)."""

    def _split_multiwaits(self):
        nc = self.nc
        blocks = nc.m.functions[0].blocks
        work = []
        for block in blocks:
            for inst in block.instructions:
                si = inst.sync_info
                if si is not None and si.on_wait and len(si.on_wait) > 1:
                    work.append(inst)
        if not work:
            return
        # Create the helper nops (they append to cur_bb; we pull them out).
        nop_map = {}
        created = []
        for inst in work:
            waits = list(inst.sync_info.on_wait)
            nops = []
            for w in waits[:-1]:
                bi = nc.engines[inst.engine].nop(nofuse=True)
                nsi = bi.ins.sync_info
                if nsi is None:
                    bi.ins.sync_info = type(inst.sync_info)(
                        on_wait=[w], on_update=[]
                    )
                else:
                    nsi.on_wait = [w]
                nops.append(bi.ins)
                created.append(bi.ins)
            inst.sync_info.on_wait = waits[-1:]
            nop_map[inst.name] = nops
        created_names = {i.name for i in created}
        for block in blocks:
            insts = block.instructions
            if not any(i.name in nop_map or i.name in created_names
                       for i in insts):
                continue
            new = []
            for inst in insts:
                if inst.name in created_names:
                    continue  # strip from wherever nop() appended it
                new.extend(nop_map.get(inst.name, ()))
                new.append(inst)
            block.instructions[:] = new

    def _drain_and_barrier(self, tick_clock, wait_clock):
        nc = self.nc
        drain_inst = nc.sync.drain()
        wait_clock.add_sem_waits(
            drain_inst.ins, ScopedClock({None: tick_clock.global_clock})
        )
        nc.all_engine_barrier()
        assert self.sems is not None
        popped = nc._tile_sem_poison_stack.pop()
        assert popped is self._sem_poison
        nc.clear_and_free_semaphores(list(self.sems.allocated().values()))
        nc.all_engine_barrier()
        self._split_multiwaits()


def build_core_kernel():
    nc = bass.Bass(
        trn_type="TRN2", target_bir_lowering=False, debug=False,
        num_devices=N_CORES,
    )
    xT = nc.dram_tensor("xT", [D, S], FP16, kind="ExternalInput").ap()
    wqkv = nc.dram_tensor("wqkv", [D, 3 * HPC * DH], FP16, kind="ExternalInput").ap()
    wproj = nc.dram_tensor("wproj", [HPC * DH, D], FP16, kind="ExternalInput").ap()
    tri = nc.dram_tensor("tri", [128, 128], FP16, kind="ExternalInput").ap()
    out = nc.dram_tensor("out", [S, D], FP32, kind="ExternalOutput").ap()

    with FixedTileContext(nc) as tc, ExitStack() as ctx:
        p_w = ctx.enter_context(tc.tile_pool(name="w", bufs=1))
        p_xt = ctx.enter_context(tc.tile_pool(name="xt", bufs=1))
        p_qk = ctx.enter_context(tc.tile_pool(name="qk", bufs=1))
        p_v = ctx.enter_context(tc.tile_pool(name="v", bufs=1))
        p_pt = ctx.enter_context(tc.tile_pool(name="pt", bufs=1))
        p_yt = ctx.enter_context(tc.tile_pool(name="yt", bufs=1))
        p_yu = ctx.enter_context(tc.tile_pool(name="yu", bufs=8))
        p_den = ctx.enter_context(tc.tile_pool(name="den", bufs=2))
        p_ob = ctx.enter_context(tc.tile_pool(name="ob", bufs=3))
        p_ps_mm = ctx.enter_context(tc.tile_pool(name="ps_mm", bufs=4, space="PSUM"))
        p_ps_s = ctx.enter_context(tc.tile_pool(name="ps_s", bufs=2, space="PSUM"))

        # ── weights / constants ────────────────────────────────────────
        w_tiles = []
        for k in range(8):
            t = p_w.tile([128, 3 * HPC * DH], FP16, tag=f"wqkv{k}", name=f"wqkv{k}")
            nc.sync.dma_start(t[:], wqkv[k * 128:(k + 1) * 128, :])
            w_tiles.append(t)
        wp_tiles = []
        for k in range(2):
            t = p_w.tile([128, D], FP16, tag=f"wp{k}", name=f"wp{k}")
            nc.sync.dma_start(t[:], wproj[k * 128:(k + 1) * 128, :])
            wp_tiles.append(t)
        tri_t = p_w.tile([128, 128], FP16, tag="tri")
        nc.sync.dma_start(tri_t[:], tri[:, :])
        ones_row = p_w.tile([1, 64], FP16, tag="ones", name="ones_row")
        nc.vector.memset(ones_row[:], 1.0)

        # ── persistent activations ─────────────────────────────────────
        # qkT[m][p, s]: m=0,1 -> qT head pairs (2h, 2h+1), m=2,3 -> kT
        qk_tiles = [p_qk.tile([128, S], FP16, tag=f"qk{m}", name=f"qk{m}")
                    for m in range(4)]
        # v_aug[J][p=j, 4*65]: per head 64 v cols + ones col (denominator)
        v_tiles = [p_v.tile([128, HPC * 65], FP16, tag=f"v{j}", name=f"v{j}")
                   for j in range(NT)]
        # yT head pairs [128 = 2 heads x 64, S]
        yt_tiles = [p_yt.tile([128, S], FP16, tag=f"yt{m}", name=f"yt{m}")
                    for m in range(2)]

        # ── phase A: qkv projection (contraction over d on partitions) ──
        # xT resident as 8 [128, 2048] fp16 tiles, DMA'd in 512-col slices
        # (descending) so the first score tiles unblock early.
        xt_tiles = []
        for k in range(8):
            t = p_xt.tile([128, S], FP16, tag=f"xt{k}", name=f"xt{k}")
            xt_tiles.append(t)
        for sc in (3, 2, 1, 0):
            for k in range(8):
                nc.sync.dma_start(
                    xt_tiles[k][:, sc * 512:(sc + 1) * 512],
                    xT[k * 128:(k + 1) * 128, sc * 512:(sc + 1) * 512],
                )

        def qk_group(m, sc):
            """One qT/kT output group: [c-tile m, s-chunk sc]."""
            ps = p_ps_mm.tile([128, 512], FP32, tag="ps_mm", name="ps_mm")
            for k in range(8):
                nc.tensor.matmul(
                    ps[:],
                    lhsT=w_tiles[k][:, m * 128:(m + 1) * 128],
                    rhs=xt_tiles[k][:, sc * 512:(sc + 1) * 512],
                    start=(k == 0), stop=(k == 7),
                )
            nc.vector.tensor_copy(
                qk_tiles[m][:, sc * 512:(sc + 1) * 512], ps[:]
            )

        def v_tile(j):
            """v[s-tile j] = xT[:, j].T @ Wv, interleaved with ones cols."""
            ps = p_ps_mm.tile([128, 256], FP32, tag="ps_mm", name="ps_mm")
            for k in range(8):
                nc.tensor.matmul(
                    ps[:],
                    lhsT=xt_tiles[k][:, j * 128:(j + 1) * 128],
                    rhs=w_tiles[k][:, 2 * HPC * DH:3 * HPC * DH],
                    start=(k == 0), stop=(k == 7),
                )
            va3 = v_tiles[j][:].rearrange("p (h c) -> p h c", c=65)
            nc.vector.memset(va3[:, :, 64:65], 1.0)
            nc.vector.tensor_copy(
                va3[:, :, 0:64],
                ps[:].rearrange("p (h c) -> p h c", c=64),
            )

        # ── scores + exp ───────────────────────────────────────────────
        # PT tiles: [128=j, width] fp16, width = S - 128*J
        pt_tiles = {}

        def st_exp(hs, J):
            """S.T + exp for head(s) hs (same pair) at j-tile J.

            With both heads of a pair, their K=64 matmuls use partition
            halves 0:64/64:128 -> different PE row groups -> run
            concurrently (row tiling). exp ops are 1024 wide (2 psum
            banks); causal diag masking runs on the idle Pool engine."""
            hp = hs[0] // 2
            qt = qk_tiles[hp]
            kt = qk_tiles[2 + hp]
            i0 = J * 128
            width = S - i0
            pts = {}
            for h in hs:
                par = h % 2
                pt = p_pt.tile([128, width], FP16, tag=f"pt{J}_{par}",
                               name=f"pt{h}_{J}")
                pt_tiles[(h, J)] = pt
                pts[h] = pt
            for t in range((width + 1023) // 1024):
                n = min(1024, width - t * 1024)
                pss2 = {}
                for h in hs:
                    ps = p_ps_s.tile([128, 1024], FP32, tag="ps_s",
                                     name="ps_s")
                    pss2[h] = ps
                for u in range((n + 511) // 512):
                    nu = min(512, n - u * 512)
                    for h in hs:
                        lo = (h % 2) * 64
                        nc.tensor.matmul(
                            pss2[h][:, u * 512:u * 512 + nu],
                            lhsT=kt[lo:lo + 64, i0:i0 + 128],
                            rhs=qt[lo:lo + 64,
                                   i0 + t * 1024 + u * 512:
                                   i0 + t * 1024 + u * 512 + nu],
                            start=True, stop=True,
                        )
                for h in hs:
                    nc.scalar.activation(
                        pts[h][:, t * 1024:t * 1024 + n],
                        pss2[h][:, 0:n], AF.Exp, scale=0.125,
                    )
                    if t == 0:
                        nc.gpsimd.tensor_mul(
                            pts[h][:, 0:128], pts[h][:, 0:128], tri_t[:]
                        )

        # ── PV (J-major) + deferred normalization ──────────────────────
        pending = []

        def flush_one():
            if pending:
                pending.pop(0)()

        def flush_all():
            while pending:
                pending.pop(0)()

        class PVState:
            """J-major PV for one head: 4 live [65, 512] psum chunks
            (pool-A slots) accumulate v_aug.T @ PT per J; a chunk completes
            at J = 4q+3: fast-approx reciprocal of its denominator row,
            stage unnormalized rows to SBUF (frees the bank), defer the
            ones-matmul broadcast + multiply."""

            def __init__(self, h, tail_cb=None):
                self.h = h
                self.tail_cb = tail_cb
                self.par = h % 2
                self.yt = yt_tiles[h // 2]
                self.denr32 = p_den.tile([1, S], FP32, tag="denr32",
                                         name=f"denr32_{h}")
                self.denr = p_den.tile([1, S], FP16, tag="denr",
                                       name=f"denr{h}")
                self.chunks = [
                    p_ps_mm.tile([65, 512], FP32, tag="ps_mm",
                                 name=f"ps_pv{h}_{q}")
                    for q in range(4)
                ]

            def step(self, J):
                h = self.h
                for q in range(J // 4, 4):
                    i0 = 512 * q
                    off = max(0, 128 * J - i0)
                    src = i0 + off - 128 * J
                    nc.tensor.matmul(
                        self.chunks[q][:, off:512],
                        lhsT=v_tiles[J][:, h * 65:(h + 1) * 65],
                        rhs=pt_tiles[(h, J)][:, src:src + 512 - off],
                        start=(J == 0), stop=(J == 4 * q + 3),
                    )
                del pt_tiles[(h, J)]
                if J >= 3 and (J - 3) % 4 == 0:
                    self._complete((J - 3) // 4)

            def _complete(self, q):
                h, par, yt = self.h, self.par, self.yt
                i0 = 512 * q
                ps = self.chunks[q]
                # Denominator reciprocal, lane-parallel: the [1, 512] row
                # would run on a single DVE lane (~3.3us); reshape through
                # DRAM-free DMA to [128, 4] so all lanes work (~0.1us).
                nc.vector.tensor_copy(
                    self.denr32[0:1, i0:i0 + 512], ps[64:65, :]
                )
                dcol = p_den.tile([128, 4], FP32, tag="dcol",
                                  name=f"dcol{h}_{q}", bufs=4)
                nc.sync.dma_start(dcol[:], self.denr32[0:1, i0:i0 + 512])
                rcol = p_den.tile([128, 4], FP16, tag="rcol",
                                  name=f"rcol{h}_{q}", bufs=4)
                with nc.allow_low_precision("softmax denom recip fp16"):
                    nc.vector.reciprocal(rcol[:], dcol[:])
                nc.sync.dma_start(self.denr[0:1, i0:i0 + 512], rcol[:])
                yu = p_yu.tile([64, 512], FP16, tag="yu", name=f"yu{h}_{q}")
                nc.vector.tensor_copy(yu[:], ps[0:64, :])
                denr, tail_cb = self.denr, self.tail_cb

                def finish():
                    psb = p_ps_s.tile([64, 512], FP32, tag="ps_s",
                                      name="ps_bc")
                    nc.tensor.matmul(
                        psb[:], lhsT=ones_row[0:1, :],
                        rhs=denr[0:1, i0:i0 + 512],
                        start=True, stop=True,
                    )
                    nc.vector.tensor_mul(
                        yt[par * 64:(par + 1) * 64, i0:i0 + 512],
                        yu[:], psb[:],
                    )
                    if tail_cb is not None:
                        tail_cb(q)

                pending.append(finish)

        # ── output projection (partial over this core's heads) ─────────
        def proj(q):
            for it in range(4 * q, 4 * q + 4):
                for n2 in range(2):
                    ps = p_ps_mm.tile([128, 512], FP32, tag="ps_mm",
                                      name="ps_mm")
                    for kc in range(2):
                        nc.tensor.matmul(
                            ps[:],
                            lhsT=yt_tiles[kc][:, it * 128:(it + 1) * 128],
                            rhs=wp_tiles[kc][:, n2 * 512:(n2 + 1) * 512],
                            start=(kc == 0), stop=(kc == 1),
                        )
                    ob = p_ob.tile([128, 512], FP32, tag="ob", name="ob")
                    nc.vector.tensor_copy(ob[:], ps[:])
                    nc.sync.dma_start(
                        out[it * 128:(it + 1) * 128,
                            n2 * 512:(n2 + 1) * 512], ob[:]
                    )

        # ── emission schedule (PE program order == emission order) ─────
        # W1: qk (descending s-chunks) woven with the head-0/1 paired
        #     scores+exp and half the v projection; ACT starts ~4us in.
        for sc in (3, 2, 1, 0):
            qk_group(0, sc)
            qk_group(2, sc)
            for J in range(4 * sc + 3, 4 * sc - 1, -1):
                st_exp((0, 1), J)
                if J < 8:
                    v_tile(J)
            qk_group(1, sc)
            qk_group(3, sc)
        # W2a: PV(0) J-major woven with head-2 scores (the PV step frees
        #      the parity-0 PT slot the score step refills) + rest of v.
        pv0 = PVState(0)
        for J in range(NT):
            pv0.step(J)
            st_exp((2,), J)
            if J < 8:
                v_tile(J + 8)
            flush_one()
        # W2b: PV(1) woven with head-3 scores
        pv1 = PVState(1)
        for J in range(NT):
            pv1.step(J)
            st_exp((3,), J)
            flush_one()
        # W2c/W2d: PV(2), PV(3) + projection tail
        pv2 = PVState(2)
        for J in range(NT):
            pv2.step(J)
            flush_one()
        flush_all()
        pv3 = PVState(3, tail_cb=proj)
        for J in range(NT):
            pv3.step(J)
            flush_one()
        flush_all()

    return nc


def make_in_maps(x, W_qkv, W_proj):
    tri = np.triu(np.ones((128, 128), dtype=np.float16))
    in_maps = []
    for c in range(N_CORES):
        b, g = c // 4, c % 4
        h0 = g * HPC
        cols = slice(h0 * DH, (h0 + HPC) * DH)
        wg = np.concatenate(
            [W_qkv[:, 0 * D:][:, cols], W_qkv[:, 1 * D:][:, cols],
             W_qkv[:, 2 * D:][:, cols]], axis=1,
        ).astype(np.float16)
        in_maps.append({
            "xT": np.ascontiguousarray(x[b].T).astype(np.float16),
            "wqkv": np.ascontiguousarray(wg),
            "wproj": np.ascontiguousarray(W_proj[cols, :]).astype(np.float16),
            "tri": tri,
        })
    return in_maps


_NC_CACHE = None


def run(x, W_qkv, W_proj, trace=False):
    global _NC_CACHE
    if _NC_CACHE is None:
        _NC_CACHE = build_core_kernel()
    nc = _NC_CACHE
    in_maps = make_in_maps(x, W_qkv, W_proj)
    res = run_bass_kernel_spmd(nc, in_maps, list(range(N_CORES)), trace=trace)
    outs = [res.results[c]["out"] for c in range(N_CORES)]
    full = np.stack(
        [outs[4 * b] + outs[4 * b + 1] + outs[4 * b + 2] + outs[4 * b + 3]
         for b in range(B)]
    ).astype(np.float32)
    return full, res


def kernel(x, W_qkv, W_proj):
    full, _ = run(np.asarray(x), np.asarray(W_qkv), np.asarray(W_proj))
    return full
